# revision 13
# baseline (speedup 1.0000x reference)
"""Trainium2 Bass kernel for the ConvMod problem:

    Y1 = valid 2x2 cross-correlation(X, W)    # [4095, 4095]
    Y2 = transposed-conv(Y1, W)               # [4096, 4096]

The composite equals, in the interior, a 3x3 convolution of X with
K = corr(W, W), plus boundary corrections from the clipping of Y1's
domain (see _make_taps).

Distribution: data-parallel over rows across 8 cores; each core gets a
[514, 4104] fp16 row slab of X with a 1-row halo on each side, plus
per-core stationary band matrices, and produces its [512, 4096] slice
of Y2.  No collectives.  All HBM I/O is fp16 (the 2e-2 rel-err budget
has ~20x margin); PSUM accumulation is fp32.

Per core the 512 output rows split into 4 blocks of M=126 rows done as
tridiagonal band matmuls (3 column-offset passes over 4096 cols each,
PSUM-accumulated per 1024-col pair tile), plus an 8-row tail computed
in a column-folded layout [12 groups x 342 cols packed on partitions]
so its 3 band passes cost 342 moving columns instead of 4096.  Edge
corrections for output columns 0 / 4095 are N=2 matmuls on staging
columns; row-boundary corrections are baked into the per-core
stationary data (SPMD: same program, different data per core).

Engine roles: sync = input DMA (left halves + small tensors),
scalar = input DMA (right halves) + 2 PSUM evacuations per block,
vector = 2 evacuations per block + tail evac, gpsimd = output DMA
(SWDGE), tensor = matmuls only.  A few warmup matmuls on a zeroed
tile at t=0 climb the PE p-state ramp (0.65 -> 2.4 GHz) while the
first input DMA is in flight.
"""

import numpy as np

import concourse.bass as bass
from concourse import bacc
import concourse.mybir as mybir
from concourse.tile import TileContext
from concourse.bass_utils import run_bass_kernel_spmd

H = 4096
L = 4096
NCORES = 8
RPC = H // NCORES          # output rows per core: 512
SLAB = RPC + 2             # input slab rows per core (1-row halo each side)
STG = 6                    # staging cols at front: [X0, 0, 0, XL, 0, 0]
LEXT = STG + L + 2         # + 2 trailing zero cols (right pad for v=+1)
M = 126                    # output rows per main block
NBLK = 4                   # main blocks per core (4*126 = 504 rows)
MAIN = NBLK * M            # 504
PAIR = 1024                # psum pair-tile columns (2 banks)
NPAIR = L // PAIR          # 4
WPAD_M = 126
NSETS = 2                  # stationary sets: 0 = block 0, 1 = blocks 1..3
# tail: rows 504..511 in column-folded layout
TG = 12                    # groups
TGW = 342                  # cols per group (12*342 = 4104 >= 4096)
TR = 10                    # input rows for the tail (slab rows 504..513)
TM = 8                     # tail output rows
TKIN = TG * TR             # 120 moving partitions
TMOUT = TG * TM            # 96 output partitions
TXF = TGW + 2 + 4          # xtail free size: 344 window + 4 staging
NWARM = 16
F32 = mybir.dt.float32
F16 = mybir.dt.float16


# ----------------------------------------------------------------------------
# Host-side stationary-matrix construction
# ----------------------------------------------------------------------------

def _make_taps(W):
    W = np.asarray(W, dtype=np.float64)
    K = np.zeros((3, 3))
    for a in range(2):
        for b in range(2):
            for c in range(2):
                for d in range(2):
                    K[a - c + 1, b - d + 1] += W[a, b] * W[c, d]
    rowtop = np.zeros(3)
    rowbot = np.zeros(3)
    for b in range(2):
        for d in range(2):
            rowtop[b - d + 1] += W[1, b] * W[1, d]
            rowbot[b - d + 1] += W[0, b] * W[0, d]
    col0 = np.zeros(3)
    colL = np.zeros(3)
    for a in range(2):
        for c in range(2):
            col0[a - c + 1] += W[a, 1] * W[c, 1]
            colL[a - c + 1] += W[a, 0] * W[c, 0]
    corners = {
        (0, 0): W[1, 1] ** 2,
        (0, 1): W[1, 0] ** 2,
        (1, 0): W[0, 1] ** 2,
        (1, 1): W[0, 0] ** 2,
    }
    return K, rowtop, rowbot, col0, colL, corners


def _build_block_mats(W, Mb, first_row_global, last_row_global):
    """[5, Mb+2, Mb]: bands for v=-1,0,+1 then negated C0, C_L corrections."""
    K3, rowtop, rowbot, col0, colL, corners = _make_taps(W)
    Kin = Mb + 2
    mats = np.zeros((5, Kin, Mb))
    for m in range(Mb):
        for u in (-1, 0, 1):
            k = m + 1 + u
            for vi, v in enumerate((-1, 0, 1)):
                mats[vi, k, m] = K3[u + 1, v + 1]
            mats[3, k, m] = -col0[u + 1]
            mats[4, k, m] = -colL[u + 1]
    if first_row_global:
        for vi, v in enumerate((-1, 0, 1)):
            mats[vi, 1, 0] -= rowtop[v + 1]
        mats[3, 1, 0] += corners[(0, 0)]
        mats[4, 1, 0] += corners[(0, 1)]
    if last_row_global:
        m = Mb - 1
        for vi, v in enumerate((-1, 0, 1)):
            mats[vi, m + 1, m] -= rowbot[v + 1]
        mats[3, m + 1, m] += corners[(1, 0)]
        mats[4, m + 1, m] += corners[(1, 1)]
    return mats


def _build_wstack(W, core):
    """Per-core stationary stack [128, 10*126] fp16 (set-major, k-major)."""
    out = np.zeros((128, NSETS, 5, WPAD_M), dtype=np.float16)
    b0 = _build_block_mats(W, M, core == 0, False)
    mid = _build_block_mats(W, M, False, False)
    for w in range(5):
        out[:128, 0, w, :M] = b0[w].astype(np.float16)
        out[:128, 1, w, :M] = mid[w].astype(np.float16)
    return out.reshape(128, NSETS * 5 * WPAD_M)


# packed [128, SM_TOT] layout: wstack | wtail | xtail (fat DMA lines)
SM_WS = NSETS * 5 * WPAD_M          # 1260
SM_WT = SM_WS + 5 * TMOUT           # 1740
SM_TOT = SM_WT + TXF                # 2088


def _build_smalls(W, core, xtail_c):
    out = np.zeros((128, SM_TOT), dtype=np.float16)
    out[:, :SM_WS] = _build_wstack(W, core)
    out[:TKIN, SM_WS:SM_WT] = _build_wtail(W, core)
    out[:TKIN, SM_WT:] = xtail_c
    return out


def _build_wtail(W, core):
    """Tail stationary [120, 5*96] fp16: folded bands + SL + SR.

    S_w[g*TR + r, g*TM + m] = b4[w, r, m] for the 3 bands; SL only at
    g=0, SR only at g=TG-1 (their staging data is zero elsewhere, but
    zero coeffs keep it safe anyway)."""
    b4 = _build_block_mats(W, TM, False, core == NCORES - 1)  # [5, 10, 8]
    out = np.zeros((TKIN, 5, TMOUT), dtype=np.float16)
    for w in range(5):
        for g in range(TG):
            if w == 3 and g != 0:
                continue
            if w == 4 and g != TG - 1:
                continue
            out[g * TR : g * TR + TR, w, g * TM : g * TM + TM] = b4[w].astype(
                np.float16
            )
    return out.reshape(TKIN, 5 * TMOUT)


def _make_slabs(X16):
    """[8, SLAB, LEXT] fp16 slabs: staging cols 0..5 then X then 2 zero."""
    slabs = np.zeros((NCORES, SLAB, LEXT), dtype=np.float16)
    for c in range(NCORES):
        lo = c * RPC - 1
        hi = c * RPC + RPC + 1
        src_lo = max(0, lo)
        src_hi = min(H, hi)
        slabs[c, src_lo - lo : src_hi - lo, STG : STG + L] = X16[src_lo:src_hi, :]
    slabs[:, :, 0] = slabs[:, :, STG]          # X0
    slabs[:, :, 3] = slabs[:, :, STG + L - 1]  # XL
    return slabs


def _make_xtail(X16):
    """[8, TKIN, TXF] fp16 folded tail input, partition p = g*TR + r."""
    xt = np.zeros((NCORES, TKIN, TXF), dtype=np.float16)
    for c in range(NCORES):
        for r in range(TR):
            gr = c * RPC + MAIN - 1 + r
            if gr >= H:
                continue
            row = X16[gr]
            for g in range(TG):
                j0 = g * TGW - 1
                a = max(0, j0)
                b = min(L, j0 + TGW + 2)
                if a < b:
                    xt[c, g * TR + r, a - j0 : b - j0] = row[a:b]
            xt[c, 0 * TR + r, TGW + 2] = row[0]       # SL staging [X0, 0]
            xt[c, (TG - 1) * TR + r, TGW + 5] = row[L - 1]  # SR staging [0, XL]
    return xt


# ----------------------------------------------------------------------------
# Device program (SPMD; identical instruction stream on all 8 cores)
# ----------------------------------------------------------------------------

def build_nc(compile=True):
    nc = bacc.Bacc()
    x_d = nc.declare_dram_parameter("xslab", [SLAB, LEXT], F16, isOutput=False)
    sm_d = nc.declare_dram_parameter("smalls", [128, SM_TOT], F16, isOutput=False)
    y_d = nc.declare_dram_parameter("y", [MAIN, L], F16, isOutput=True)
    yt_d = nc.declare_dram_parameter("ytail", [TMOUT, TGW], F16, isOutput=True)

    with TileContext(nc) as tc:
        with (
            tc.tile_pool(name="wp", bufs=1) as wp,
            tc.tile_pool(name="xp", bufs=4) as xp,
            tc.tile_pool(name="yp", bufs=4) as yp,
            tc.tile_pool(name="pp", bufs=3, space="PSUM") as pp,
            tc.tile_pool(name="pt", bufs=1, space="PSUM") as ppt,
            tc.tile_pool(name="pw", bufs=1, space="PSUM") as ppw,
        ):
            smsb = wp.tile([128, SM_TOT], F16, name="smsb")
            wsb = smsb
            wz = wp.tile([128, 512], F16, name="wz")
            ytlsb = wp.tile([TMOUT, TGW], F16, name="ytlsb")

            # -- input DMA triggers, all up front (queues stream ahead) --
            xts = [
                xp.tile([128, LEXT], F16, name=f"xt{b}", tag="xt")
                for b in range(NBLK)
            ]
            # L half covers staging + X cols up to pair-1's v=+1 reach, so
            # pairs 0,1 of a block depend only on the L piece
            HALF = STG + 2050
            # sync's queue spins up ~2us before scalar's (scalar's first
            # trigger sits behind the hoisted ACT_TABLE_LOAD), so ALL the
            # gating tensors ride sync, interleaved for earliest pair-0
            # start: B0L, then weights, then B0R, then later L halves
            nc.sync.dma_start(out=xts[0][:, 0:HALF], in_=x_d[0:128, 0:HALF])
            nc.sync.dma_start(out=smsb[:, :], in_=sm_d[:, :])
            nc.sync.dma_start(
                out=xts[0][:, HALF:LEXT], in_=x_d[0:128, HALF:LEXT]
            )
            for b in range(1, NBLK):
                s = b * M
                nc.sync.dma_start(
                    out=xts[b][:, 0:HALF], in_=x_d[s : s + 128, 0:HALF]
                )
                nc.scalar.dma_start(
                    out=xts[b][:, HALF:LEXT], in_=x_d[s : s + 128, HALF:LEXT]
                )

            # -- PE warmup (p-state ramp); results are discarded --
            nc.vector.memset(wz[:, :], 0.0)
            pw = ppw.tile([128, 512], F32, name="pwarm")
            for i in range(NWARM):
                nc.tensor.matmul(
                    pw[0:128, 0:512], wz[0:128, 0:128], wz[0:128, 0:512],
                    start=True, stop=True,
                )

            def twm(w):
                c0 = SM_WS + w * TMOUT
                return smsb[0:TKIN, c0 : c0 + TMOUT]

            def txr(off, n):
                return smsb[0:TKIN, SM_WT + off : SM_WT + off + n]

            def do_tail():
                # column-folded tail; data rides in the packed smalls
                ptl = ppt.tile([128, TGW], F32, name="ptail")
                nc.tensor.matmul(
                    ptl[0:TMOUT, 0:TGW], twm(0), txr(0, TGW),
                    start=True, stop=False,
                )
                nc.tensor.matmul(
                    ptl[0:TMOUT, 0:TGW], twm(1), txr(1, TGW),
                    start=False, stop=False,
                )
                nc.tensor.matmul(
                    ptl[0:TMOUT, 0:TGW], twm(2), txr(2, TGW),
                    start=False, stop=False,
                )
                nc.tensor.matmul(
                    ptl[0:TMOUT, 0:2], twm(3), txr(TGW + 2, 2),
                    start=False, stop=False,
                )
                nc.tensor.matmul(
                    ptl[0:TMOUT, TGW - 10 : TGW - 8], twm(4), txr(TGW + 4, 2),
                    start=False, stop=True,
                )
                nc.vector.tensor_copy(ytlsb[0:TMOUT, 0:TGW], ptl[0:TMOUT, 0:TGW])
                nc.gpsimd.dma_start(out=yt_d[:, :], in_=ytlsb[0:TMOUT, 0:TGW])

            # -- main blocks (tail slots in after block 0) --
            for b in range(NBLK):
                s = b * M
                si = 0 if b == 0 else 1
                xt = xts[b]
                yt = yp.tile([128, L], F16, name=f"yt{b}", tag="yt")

                def wm(w):
                    base = (si * 5 + w) * WPAD_M
                    return wsb[0:128, base : base + M]

                def xr(c0, n):
                    # moving slice for X cols [c0, c0+n) (c0 may be -1 to
                    # touch the staged zero at tile col 5)
                    return xt[0:128, STG + c0 : STG + c0 + n]

                for k in range(NPAIR):
                    pt = pp.tile([128, PAIR], F32, name=f"pt{b}_{k}", tag="pt")
                    cA = k * PAIR
                    cB = cA + 512
                    # band v=0 (start clears each psum half)
                    nc.tensor.matmul(
                        pt[0:M, 0:512], wm(1), xr(cA, 512), start=True, stop=False
                    )
                    nc.tensor.matmul(
                        pt[0:M, 512:1024], wm(1), xr(cB, 512), start=True, stop=False
                    )
                    # band v=-1
                    nc.tensor.matmul(
                        pt[0:M, 0:512], wm(0), xr(cA - 1, 512),
                        start=False, stop=False,
                    )
                    nc.tensor.matmul(
                        pt[0:M, 512:1024], wm(0), xr(cB - 1, 512),
                        start=False, stop=False,
                    )
                    # band v=+1 (+ edge fixes on the outermost pairs)
                    nc.tensor.matmul(
                        pt[0:M, 0:512], wm(2), xr(cA + 1, 512),
                        start=False, stop=(k != 0),
                    )
                    if k == 0:
                        # col 0 fix: moving staging [X0, 0] at tile cols 0:2
                        nc.tensor.matmul(
                            pt[0:M, 0:2], wm(3), xt[0:128, 0:2],
                            start=False, stop=True,
                        )
                    nc.tensor.matmul(
                        pt[0:M, 512:1024], wm(2), xr(cB + 1, 512),
                        start=False, stop=(k != NPAIR - 1),
                    )
                    if k == NPAIR - 1:
                        # col L-1 fix: moving staging [0, XL] at tile cols 2:4
                        nc.tensor.matmul(
                            pt[0:M, 1022:1024], wm(4), xt[0:128, 2:4],
                            start=False, stop=True,
                        )
                    # evacuate pair to SBUF (fp32 -> fp16); the very last
                    # pair is split across both engines for tail latency
                    if b == NBLK - 1 and k == NPAIR - 1:
                        nc.scalar.copy(
                            yt[0:M, cA : cA + 512], pt[0:M, 0:512]
                        )
                        nc.vector.tensor_copy(
                            yt[0:M, cB : cB + 512], pt[0:M, 512:1024]
                        )
                        nc.scalar.dma_start(
                            out=y_d[s : s + M, cA : cA + 512],
                            in_=yt[0:M, cA : cA + 512],
                        )
                        nc.sync.dma_start(
                            out=y_d[s : s + M, cB : cB + 512],
                            in_=yt[0:M, cB : cB + 512],
                        )
                    else:
                        dst = yt[0:M, cA : cA + PAIR]
                        src = pt[0:M, 0:PAIR]
                        if k % 2 == 0:
                            nc.scalar.copy(dst, src)
                        else:
                            nc.vector.tensor_copy(dst, src)
                        if b == NBLK - 1:
                            # stream the last block's quarters out early,
                            # alternating queues
                            q_eng = nc.scalar if k % 2 == 0 else nc.sync
                            q_eng.dma_start(
                                out=y_d[s : s + M, cA : cA + PAIR],
                                in_=yt[0:M, cA : cA + PAIR],
                            )
                    # output DMA per block at k==3 below
                    if False:
                        pass
                    elif k == 3 and b < NBLK - 1:
                        # full-block output on the (warmed) hard queues;
                        # scalar carries more output since sync carries
                        # most of the input
                        if b == 0:
                            nc.scalar.dma_start(
                                out=y_d[s : s + M, 0:L], in_=yt[0:M, 0:L]
                            )
                        elif b == 1:
                            nc.scalar.dma_start(
                                out=y_d[s : s + M, 0:L], in_=yt[0:M, 0:L]
                            )
                        else:
                            nc.sync.dma_start(
                                out=y_d[s : s + M, 0:L], in_=yt[0:M, 0:L]
                            )
                if b == 0:
                    do_tail()
    if compile:
        nc.compile()
    return nc


_NC_CACHE = None


def _get_nc():
    global _NC_CACHE
    if _NC_CACHE is None:
        _NC_CACHE = build_nc()
    return _NC_CACHE


def _run(X, W, trace=False, **spmd_kwargs):
    X16 = np.asarray(X, dtype=np.float16)
    slabs = _make_slabs(X16)
    xtails = _make_xtail(X16)
    in_maps = []
    for c in range(NCORES):
        in_maps.append(
            {
                "xslab": slabs[c],
                "smalls": _build_smalls(W, c, xtails[c]),
            }
        )
    res = run_bass_kernel_spmd(
        _get_nc(), in_maps, core_ids=list(range(NCORES)), trace=trace, **spmd_kwargs
    )
    Y = np.empty((H, L), dtype=np.float32)
    for c in range(NCORES):
        r0 = c * RPC
        Y[r0 : r0 + MAIN] = res.results[c]["y"].astype(np.float32)
        ytl = res.results[c]["ytail"].astype(np.float32)  # [96, 342]
        tail = ytl.reshape(TG, TM, TGW).transpose(1, 0, 2).reshape(TM, TG * TGW)
        Y[r0 + MAIN : r0 + RPC] = tail[:, :L]
    return Y, res


def kernel(X, W):
    Y, _ = _run(X, W)
    return Y


# revision 15
# speedup vs baseline: 1.0946x; 1.0946x over previous
"""Trainium2 Bass kernel for the ConvMod problem:

    Y1 = valid 2x2 cross-correlation(X, W)    # [4095, 4095]
    Y2 = transposed-conv(Y1, W)               # [4096, 4096]

The composite equals, in the interior, a 3x3 convolution of X with
K = corr(W, W), plus boundary corrections from the clipping of Y1's
domain (see _make_taps).

Distribution: data-parallel over rows across 8 cores; each core gets a
[514, 4104] fp16 row slab of X with a 1-row halo on each side, plus
per-core stationary band matrices, and produces its [512, 4096] slice
of Y2.  No collectives.  All HBM I/O is fp16 (the 2e-2 rel-err budget
has ~20x margin); PSUM accumulation is fp32.

Per core the 512 output rows split into 4 blocks of M=126 rows done as
tridiagonal band matmuls (3 column-offset passes over 4096 cols each,
PSUM-accumulated per 1024-col pair tile), plus an 8-row tail computed
in a column-folded layout [12 groups x 342 cols packed on partitions]
so its 3 band passes cost 342 moving columns instead of 4096.  Edge
corrections for output columns 0 / 4095 are N=2 matmuls on staging
columns; row-boundary corrections are baked into the per-core
stationary data (SPMD: same program, different data per core).

Engine roles: sync = input DMA (left halves + small tensors),
scalar = input DMA (right halves) + 2 PSUM evacuations per block,
vector = 2 evacuations per block + tail evac, gpsimd = output DMA
(SWDGE), tensor = matmuls only.  A few warmup matmuls on a zeroed
tile at t=0 climb the PE p-state ramp (0.65 -> 2.4 GHz) while the
first input DMA is in flight.
"""

import numpy as np

import concourse.bass as bass
from concourse import bacc
import concourse.mybir as mybir
from concourse.tile import TileContext
from concourse.bass_utils import run_bass_kernel_spmd

H = 4096
L = 4096
NCORES = 8
RPC = H // NCORES          # output rows per core: 512
SLAB = RPC + 2             # input slab rows per core (1-row halo each side)
STG = 6                    # staging cols at front: [X0, 0, 0, XL, 0, 0]
LEXT = STG + L + 2         # + 2 trailing zero cols (right pad for v=+1)
M = 126                    # output rows per main block
NBLK = 4                   # main blocks per core (4*126 = 504 rows)
MAIN = NBLK * M            # 504
PAIR = 1024                # psum pair-tile columns (2 banks)
NPAIR = L // PAIR          # 4
WPAD_M = 126
NSETS = 2                  # stationary sets: 0 = block 0, 1 = blocks 1..3
# tail: rows 504..511 in column-folded layout
TG = 12                    # groups
TGW = 342                  # cols per group (12*342 = 4104 >= 4096)
TR = 10                    # input rows for the tail (slab rows 504..513)
TM = 8                     # tail output rows
TKIN = TG * TR             # 120 moving partitions
TMOUT = TG * TM            # 96 output partitions
TXF = TGW + 2 + 4          # xtail free size: 344 window + 4 staging
NWARM = 13
F32 = mybir.dt.float32
F16 = mybir.dt.float16


# ----------------------------------------------------------------------------
# Host-side stationary-matrix construction
# ----------------------------------------------------------------------------

def _make_taps(W):
    W = np.asarray(W, dtype=np.float64)
    K = np.zeros((3, 3))
    for a in range(2):
        for b in range(2):
            for c in range(2):
                for d in range(2):
                    K[a - c + 1, b - d + 1] += W[a, b] * W[c, d]
    rowtop = np.zeros(3)
    rowbot = np.zeros(3)
    for b in range(2):
        for d in range(2):
            rowtop[b - d + 1] += W[1, b] * W[1, d]
            rowbot[b - d + 1] += W[0, b] * W[0, d]
    col0 = np.zeros(3)
    colL = np.zeros(3)
    for a in range(2):
        for c in range(2):
            col0[a - c + 1] += W[a, 1] * W[c, 1]
            colL[a - c + 1] += W[a, 0] * W[c, 0]
    corners = {
        (0, 0): W[1, 1] ** 2,
        (0, 1): W[1, 0] ** 2,
        (1, 0): W[0, 1] ** 2,
        (1, 1): W[0, 0] ** 2,
    }
    return K, rowtop, rowbot, col0, colL, corners


def _build_block_mats(W, Mb, first_row_global, last_row_global):
    """[5, Mb+2, Mb]: bands for v=-1,0,+1 then negated C0, C_L corrections."""
    K3, rowtop, rowbot, col0, colL, corners = _make_taps(W)
    Kin = Mb + 2
    mats = np.zeros((5, Kin, Mb))
    for m in range(Mb):
        for u in (-1, 0, 1):
            k = m + 1 + u
            for vi, v in enumerate((-1, 0, 1)):
                mats[vi, k, m] = K3[u + 1, v + 1]
            mats[3, k, m] = -col0[u + 1]
            mats[4, k, m] = -colL[u + 1]
    if first_row_global:
        for vi, v in enumerate((-1, 0, 1)):
            mats[vi, 1, 0] -= rowtop[v + 1]
        mats[3, 1, 0] += corners[(0, 0)]
        mats[4, 1, 0] += corners[(0, 1)]
    if last_row_global:
        m = Mb - 1
        for vi, v in enumerate((-1, 0, 1)):
            mats[vi, m + 1, m] -= rowbot[v + 1]
        mats[3, m + 1, m] += corners[(1, 0)]
        mats[4, m + 1, m] += corners[(1, 1)]
    return mats


def _build_wstack(W, core):
    """Per-core stationary stack [128, 10*126] fp16 (set-major, k-major)."""
    out = np.zeros((128, NSETS, 5, WPAD_M), dtype=np.float16)
    b0 = _build_block_mats(W, M, core == 0, False)
    mid = _build_block_mats(W, M, False, False)
    for w in range(5):
        out[:128, 0, w, :M] = b0[w].astype(np.float16)
        out[:128, 1, w, :M] = mid[w].astype(np.float16)
    return out.reshape(128, NSETS * 5 * WPAD_M)


# packed [128, SM_TOT] layout: wstack | wtail | xtail (fat DMA lines)
SM_WS = NSETS * 5 * WPAD_M          # 1260
SM_WT = SM_WS + 5 * TMOUT           # 1740
SM_TOT = SM_WT + TXF                # 2088


def _build_smalls(W, core, xtail_c):
    out = np.zeros((128, SM_TOT), dtype=np.float16)
    out[:, :SM_WS] = _build_wstack(W, core)
    out[:TKIN, SM_WS:SM_WT] = _build_wtail(W, core)
    out[:TKIN, SM_WT:] = xtail_c
    return out


def _build_wtail(W, core):
    """Tail stationary [120, 5*96] fp16: folded bands + SL + SR.

    S_w[g*TR + r, g*TM + m] = b4[w, r, m] for the 3 bands; SL only at
    g=0, SR only at g=TG-1 (their staging data is zero elsewhere, but
    zero coeffs keep it safe anyway)."""
    b4 = _build_block_mats(W, TM, False, core == NCORES - 1)  # [5, 10, 8]
    out = np.zeros((TKIN, 5, TMOUT), dtype=np.float16)
    for w in range(5):
        for g in range(TG):
            if w == 3 and g != 0:
                continue
            if w == 4 and g != TG - 1:
                continue
            out[g * TR : g * TR + TR, w, g * TM : g * TM + TM] = b4[w].astype(
                np.float16
            )
    return out.reshape(TKIN, 5 * TMOUT)


def _make_slabs(X16):
    """[8, SLAB, LEXT] fp16 slabs: staging cols 0..5 then X then 2 zero."""
    slabs = np.zeros((NCORES, SLAB, LEXT), dtype=np.float16)
    for c in range(NCORES):
        lo = c * RPC - 1
        hi = c * RPC + RPC + 1
        src_lo = max(0, lo)
        src_hi = min(H, hi)
        slabs[c, src_lo - lo : src_hi - lo, STG : STG + L] = X16[src_lo:src_hi, :]
    slabs[:, :, 0] = slabs[:, :, STG]          # X0
    slabs[:, :, 3] = slabs[:, :, STG + L - 1]  # XL
    return slabs


def _make_xtail(X16):
    """[8, TKIN, TXF] fp16 folded tail input, partition p = g*TR + r."""
    xt = np.zeros((NCORES, TKIN, TXF), dtype=np.float16)
    for c in range(NCORES):
        for r in range(TR):
            gr = c * RPC + MAIN - 1 + r
            if gr >= H:
                continue
            row = X16[gr]
            for g in range(TG):
                j0 = g * TGW - 1
                a = max(0, j0)
                b = min(L, j0 + TGW + 2)
                if a < b:
                    xt[c, g * TR + r, a - j0 : b - j0] = row[a:b]
            xt[c, 0 * TR + r, TGW + 2] = row[0]       # SL staging [X0, 0]
            xt[c, (TG - 1) * TR + r, TGW + 5] = row[L - 1]  # SR staging [0, XL]
    return xt


# ----------------------------------------------------------------------------
# Device program (SPMD; identical instruction stream on all 8 cores)
# ----------------------------------------------------------------------------

def build_nc(compile=True):
    nc = bacc.Bacc()
    x_d = nc.declare_dram_parameter("xslab", [SLAB, LEXT], F16, isOutput=False)
    sm_d = nc.declare_dram_parameter("smalls", [128, SM_TOT], F16, isOutput=False)
    y_d = nc.declare_dram_parameter("y", [MAIN, L], F16, isOutput=True)
    yt_d = nc.declare_dram_parameter("ytail", [TMOUT, TGW], F16, isOutput=True)

    with TileContext(nc) as tc:
        with (
            tc.tile_pool(name="wp", bufs=1) as wp,
            tc.tile_pool(name="xp", bufs=4) as xp,
            tc.tile_pool(name="yp", bufs=4) as yp,
            tc.tile_pool(name="pp", bufs=3, space="PSUM") as pp,
            tc.tile_pool(name="pt", bufs=1, space="PSUM") as ppt,
            tc.tile_pool(name="pw", bufs=1, space="PSUM") as ppw,
        ):
            smsb = wp.tile([128, SM_TOT], F16, name="smsb")
            wsb = smsb
            wz = wp.tile([128, 512], F16, name="wz")
            ytlsb = wp.tile([TMOUT, TGW], F16, name="ytlsb")

            # -- input DMA triggers, all up front (queues stream ahead) --
            xts = [
                xp.tile([128, LEXT], F16, name=f"xt{b}", tag="xt")
                for b in range(NBLK)
            ]
            # L half covers staging + X cols up to pair-1's v=+1 reach, so
            # pairs 0,1 of a block depend only on the L piece
            HALF = STG + 2050
            # Early per-queue DMA rate is only ~100-180 GB/s (ramping), and
            # scalar's queue starts ~2us after sync's.  Gating bytes for the
            # first matmuls (set-0 weights + the A-quarter of block 0) ride
            # sync first, in fine grains; everything else splits across both
            # queues by deadline.
            QA = STG + 1026  # covers pair-0 A-chunk reads (cols <= STG+1025)
            SET0 = 5 * WPAD_M
            nc.sync.dma_start(out=xts[0][:, 0:QA], in_=x_d[0:128, 0:QA])
            nc.sync.dma_start(out=smsb[:, 0:SET0], in_=sm_d[:, 0:SET0])
            nc.sync.dma_start(out=xts[0][:, QA:HALF], in_=x_d[0:128, QA:HALF])
            nc.sync.dma_start(
                out=xts[0][:, HALF:LEXT], in_=x_d[0:128, HALF:LEXT]
            )
            nc.scalar.dma_start(out=smsb[:, SET0:], in_=sm_d[:, SET0:])
            for b in range(1, NBLK):
                s = b * M
                nc.sync.dma_start(
                    out=xts[b][:, 0:HALF], in_=x_d[s : s + 128, 0:HALF]
                )
                nc.scalar.dma_start(
                    out=xts[b][:, HALF:LEXT], in_=x_d[s : s + 128, HALF:LEXT]
                )

            # -- PE warmup (p-state ramp); results are discarded --
            nc.vector.memset(wz[:, :], 0.0)
            pw = ppw.tile([128, 512], F32, name="pwarm")
            for i in range(NWARM):
                nc.tensor.matmul(
                    pw[0:128, 0:512], wz[0:128, 0:128], wz[0:128, 0:512],
                    start=True, stop=True,
                )

            def twm(w):
                c0 = SM_WS + w * TMOUT
                return smsb[0:TKIN, c0 : c0 + TMOUT]

            def txr(off, n):
                return smsb[0:TKIN, SM_WT + off : SM_WT + off + n]

            def do_tail():
                # column-folded tail; data rides in the packed smalls
                ptl = ppt.tile([128, TGW], F32, name="ptail")
                nc.tensor.matmul(
                    ptl[0:TMOUT, 0:TGW], twm(0), txr(0, TGW),
                    start=True, stop=False,
                )
                nc.tensor.matmul(
                    ptl[0:TMOUT, 0:TGW], twm(1), txr(1, TGW),
                    start=False, stop=False,
                )
                nc.tensor.matmul(
                    ptl[0:TMOUT, 0:TGW], twm(2), txr(2, TGW),
                    start=False, stop=False,
                )
                nc.tensor.matmul(
                    ptl[0:TMOUT, 0:2], twm(3), txr(TGW + 2, 2),
                    start=False, stop=False,
                )
                nc.tensor.matmul(
                    ptl[0:TMOUT, TGW - 10 : TGW - 8], twm(4), txr(TGW + 4, 2),
                    start=False, stop=True,
                )
                nc.vector.tensor_copy(ytlsb[0:TMOUT, 0:TGW], ptl[0:TMOUT, 0:TGW])
                nc.gpsimd.dma_start(out=yt_d[:, :], in_=ytlsb[0:TMOUT, 0:TGW])

            # -- main blocks (tail slots in after block 0) --
            for b in range(NBLK):
                s = b * M
                si = 0 if b == 0 else 1
                xt = xts[b]
                yt = yp.tile([128, L], F16, name=f"yt{b}", tag="yt")

                def wm(w):
                    base = (si * 5 + w) * WPAD_M
                    return wsb[0:128, base : base + M]

                def xr(c0, n):
                    # moving slice for X cols [c0, c0+n) (c0 may be -1 to
                    # touch the staged zero at tile col 5)
                    return xt[0:128, STG + c0 : STG + c0 + n]

                for k in range(NPAIR):
                    pt = pp.tile([128, PAIR], F32, name=f"pt{b}_{k}", tag="pt")
                    cA = k * PAIR
                    cB = cA + 512
                    a_first = b == 0 and k == 0
                    if a_first:
                        # start on the A chunk alone: it depends only on the
                        # very first input piece
                        nc.tensor.matmul(
                            pt[0:M, 0:512], wm(1), xr(cA, 512),
                            start=True, stop=False,
                        )
                        nc.tensor.matmul(
                            pt[0:M, 0:512], wm(0), xr(cA - 1, 512),
                            start=False, stop=False,
                        )
                        nc.tensor.matmul(
                            pt[0:M, 0:512], wm(2), xr(cA + 1, 512),
                            start=False, stop=False,
                        )
                        nc.tensor.matmul(
                            pt[0:M, 0:2], wm(3), xt[0:128, 0:2],
                            start=False, stop=True,
                        )
                        nc.tensor.matmul(
                            pt[0:M, 512:1024], wm(1), xr(cB, 512),
                            start=True, stop=False,
                        )
                        nc.tensor.matmul(
                            pt[0:M, 512:1024], wm(0), xr(cB - 1, 512),
                            start=False, stop=False,
                        )
                        nc.tensor.matmul(
                            pt[0:M, 512:1024], wm(2), xr(cB + 1, 512),
                            start=False, stop=True,
                        )
                        continue_evac = True
                    else:
                        # band v=0 (start clears each psum half)
                        nc.tensor.matmul(
                            pt[0:M, 0:512], wm(1), xr(cA, 512),
                            start=True, stop=False,
                        )
                        nc.tensor.matmul(
                            pt[0:M, 512:1024], wm(1), xr(cB, 512),
                            start=True, stop=False,
                        )
                        # band v=-1
                        nc.tensor.matmul(
                            pt[0:M, 0:512], wm(0), xr(cA - 1, 512),
                            start=False, stop=False,
                        )
                        nc.tensor.matmul(
                            pt[0:M, 512:1024], wm(0), xr(cB - 1, 512),
                            start=False, stop=False,
                        )
                        # band v=+1 (+ edge fixes on the outermost pairs)
                        nc.tensor.matmul(
                            pt[0:M, 0:512], wm(2), xr(cA + 1, 512),
                            start=False, stop=(k != 0),
                        )
                        if k == 0:
                            # col 0 fix: staging [X0, 0] at tile cols 0:2
                            nc.tensor.matmul(
                                pt[0:M, 0:2], wm(3), xt[0:128, 0:2],
                                start=False, stop=True,
                            )
                        nc.tensor.matmul(
                            pt[0:M, 512:1024], wm(2), xr(cB + 1, 512),
                            start=False, stop=(k != NPAIR - 1),
                        )
                        if k == NPAIR - 1:
                            # col L-1 fix: staging [0, XL] at tile cols 2:4
                            nc.tensor.matmul(
                                pt[0:M, 1022:1024], wm(4), xt[0:128, 2:4],
                                start=False, stop=True,
                            )
                    # evacuate pair to SBUF (fp32 -> fp16); the very last
                    # pair is split across both engines for tail latency
                    if b == NBLK - 1 and k == NPAIR - 1:
                        nc.scalar.copy(
                            yt[0:M, cA : cA + 512], pt[0:M, 0:512]
                        )
                        nc.vector.tensor_copy(
                            yt[0:M, cB : cB + 512], pt[0:M, 512:1024]
                        )
                        nc.scalar.dma_start(
                            out=y_d[s : s + M, cA : cA + 512],
                            in_=yt[0:M, cA : cA + 512],
                        )
                        nc.sync.dma_start(
                            out=y_d[s : s + M, cB : cB + 512],
                            in_=yt[0:M, cB : cB + 512],
                        )
                    else:
                        dst = yt[0:M, cA : cA + PAIR]
                        src = pt[0:M, 0:PAIR]
                        if k % 2 == 0:
                            nc.scalar.copy(dst, src)
                        else:
                            nc.vector.tensor_copy(dst, src)
                        if b == NBLK - 1:
                            # stream the last block's quarters out early,
                            # alternating queues
                            q_eng = nc.scalar if k % 2 == 0 else nc.sync
                            q_eng.dma_start(
                                out=y_d[s : s + M, cA : cA + PAIR],
                                in_=yt[0:M, cA : cA + PAIR],
                            )
                    # output DMA per block at k==3 below
                    if False:
                        pass
                    elif k == 3 and b < NBLK - 1:
                        # full-block output on the (warmed) hard queues;
                        # scalar carries more output since sync carries
                        # most of the input
                        if b == 0:
                            nc.scalar.dma_start(
                                out=y_d[s : s + M, 0:L], in_=yt[0:M, 0:L]
                            )
                        elif b == 1:
                            nc.scalar.dma_start(
                                out=y_d[s : s + M, 0:L], in_=yt[0:M, 0:L]
                            )
                        else:
                            nc.sync.dma_start(
                                out=y_d[s : s + M, 0:L], in_=yt[0:M, 0:L]
                            )
                if b == 0:
                    do_tail()
    if compile:
        nc.compile()
    return nc


_NC_CACHE = None


def _get_nc():
    global _NC_CACHE
    if _NC_CACHE is None:
        _NC_CACHE = build_nc()
    return _NC_CACHE


def _run(X, W, trace=False, **spmd_kwargs):
    X16 = np.asarray(X, dtype=np.float16)
    slabs = _make_slabs(X16)
    xtails = _make_xtail(X16)
    in_maps = []
    for c in range(NCORES):
        in_maps.append(
            {
                "xslab": slabs[c],
                "smalls": _build_smalls(W, c, xtails[c]),
            }
        )
    res = run_bass_kernel_spmd(
        _get_nc(), in_maps, core_ids=list(range(NCORES)), trace=trace, **spmd_kwargs
    )
    Y = np.empty((H, L), dtype=np.float32)
    for c in range(NCORES):
        r0 = c * RPC
        Y[r0 : r0 + MAIN] = res.results[c]["y"].astype(np.float32)
        ytl = res.results[c]["ytail"].astype(np.float32)  # [96, 342]
        tail = ytl.reshape(TG, TM, TGW).transpose(1, 0, 2).reshape(TM, TG * TGW)
        Y[r0 + MAIN : r0 + RPC] = tail[:, :L]
    return Y, res


def kernel(X, W):
    Y, _ = _run(X, W)
    return Y


# revision 17
# speedup vs baseline: 1.1172x; 1.0207x over previous
"""Trainium2 Bass kernel for the ConvMod problem:

    Y1 = valid 2x2 cross-correlation(X, W)    # [4095, 4095]
    Y2 = transposed-conv(Y1, W)               # [4096, 4096]

The composite equals, in the interior, a 3x3 convolution of X with
K = corr(W, W), plus boundary corrections from the clipping of Y1's
domain (see _make_taps).

Distribution: data-parallel over rows across 8 cores; each core gets a
[514, 4104] fp16 row slab of X with a 1-row halo on each side, plus
per-core stationary band matrices, and produces its [512, 4096] slice
of Y2.  No collectives.  All HBM I/O is fp16 (the 2e-2 rel-err budget
has ~20x margin); PSUM accumulation is fp32.

Per core the 512 output rows split into 4 blocks of M=126 rows done as
tridiagonal band matmuls (3 column-offset passes over 4096 cols each,
PSUM-accumulated per 1024-col pair tile), plus an 8-row tail computed
in a column-folded layout [12 groups x 342 cols packed on partitions]
so its 3 band passes cost 342 moving columns instead of 4096.  Edge
corrections for output columns 0 / 4095 are N=2 matmuls on staging
columns; row-boundary corrections are baked into the per-core
stationary data (SPMD: same program, different data per core).

Engine roles: sync = input DMA (left halves + small tensors),
scalar = input DMA (right halves) + 2 PSUM evacuations per block,
vector = 2 evacuations per block + tail evac, gpsimd = output DMA
(SWDGE), tensor = matmuls only.  A few warmup matmuls on a zeroed
tile at t=0 climb the PE p-state ramp (0.65 -> 2.4 GHz) while the
first input DMA is in flight.
"""

import numpy as np

import concourse.bass as bass
from concourse import bacc
import concourse.mybir as mybir
from concourse.tile import TileContext
from concourse.bass_utils import run_bass_kernel_spmd

H = 4096
L = 4096
NCORES = 8
RPC = H // NCORES          # output rows per core: 512
SLAB = RPC + 2             # input slab rows per core (1-row halo each side)
STG = 6                    # staging cols at front: [X0, 0, 0, XL, 0, 0]
LEXT = STG + L + 2         # + 2 trailing zero cols (right pad for v=+1)
M = 126                    # output rows per main block
NBLK = 4                   # main blocks per core (4*126 = 504 rows)
MAIN = NBLK * M            # 504
PAIR = 1024                # psum pair-tile columns (2 banks)
NPAIR = L // PAIR          # 4
WPAD_M = 126
NSETS = 2                  # stationary sets: 0 = block 0, 1 = blocks 1..3
# tail: rows 504..511 in column-folded layout
TG = 12                    # groups
TGW = 342                  # cols per group (12*342 = 4104 >= 4096)
TR = 10                    # input rows for the tail (slab rows 504..513)
TM = 8                     # tail output rows
TKIN = TG * TR             # 120 moving partitions
TMOUT = TG * TM            # 96 output partitions
TXF = TGW + 2 + 4          # xtail free size: 344 window + 4 staging
NWARM = 12
F32 = mybir.dt.float32
F16 = mybir.dt.float16


# ----------------------------------------------------------------------------
# Host-side stationary-matrix construction
# ----------------------------------------------------------------------------

def _make_taps(W):
    W = np.asarray(W, dtype=np.float64)
    K = np.zeros((3, 3))
    for a in range(2):
        for b in range(2):
            for c in range(2):
                for d in range(2):
                    K[a - c + 1, b - d + 1] += W[a, b] * W[c, d]
    rowtop = np.zeros(3)
    rowbot = np.zeros(3)
    for b in range(2):
        for d in range(2):
            rowtop[b - d + 1] += W[1, b] * W[1, d]
            rowbot[b - d + 1] += W[0, b] * W[0, d]
    col0 = np.zeros(3)
    colL = np.zeros(3)
    for a in range(2):
        for c in range(2):
            col0[a - c + 1] += W[a, 1] * W[c, 1]
            colL[a - c + 1] += W[a, 0] * W[c, 0]
    corners = {
        (0, 0): W[1, 1] ** 2,
        (0, 1): W[1, 0] ** 2,
        (1, 0): W[0, 1] ** 2,
        (1, 1): W[0, 0] ** 2,
    }
    return K, rowtop, rowbot, col0, colL, corners


def _build_block_mats(W, Mb, first_row_global, last_row_global):
    """[5, Mb+2, Mb]: bands for v=-1,0,+1 then negated C0, C_L corrections."""
    K3, rowtop, rowbot, col0, colL, corners = _make_taps(W)
    Kin = Mb + 2
    mats = np.zeros((5, Kin, Mb))
    for m in range(Mb):
        for u in (-1, 0, 1):
            k = m + 1 + u
            for vi, v in enumerate((-1, 0, 1)):
                mats[vi, k, m] = K3[u + 1, v + 1]
            mats[3, k, m] = -col0[u + 1]
            mats[4, k, m] = -colL[u + 1]
    if first_row_global:
        for vi, v in enumerate((-1, 0, 1)):
            mats[vi, 1, 0] -= rowtop[v + 1]
        mats[3, 1, 0] += corners[(0, 0)]
        mats[4, 1, 0] += corners[(0, 1)]
    if last_row_global:
        m = Mb - 1
        for vi, v in enumerate((-1, 0, 1)):
            mats[vi, m + 1, m] -= rowbot[v + 1]
        mats[3, m + 1, m] += corners[(1, 0)]
        mats[4, m + 1, m] += corners[(1, 1)]
    return mats


def _build_wstack(W, core):
    """Per-core stationary stack [128, 10*126] fp16 (set-major, k-major)."""
    out = np.zeros((128, NSETS, 5, WPAD_M), dtype=np.float16)
    b0 = _build_block_mats(W, M, core == 0, False)
    mid = _build_block_mats(W, M, False, False)
    for w in range(5):
        out[:128, 0, w, :M] = b0[w].astype(np.float16)
        out[:128, 1, w, :M] = mid[w].astype(np.float16)
    return out.reshape(128, NSETS * 5 * WPAD_M)


# packed [128, SM_TOT] layout: wstack | wtail | xtail (fat DMA lines)
SM_WS = NSETS * 5 * WPAD_M          # 1260
SM_WT = SM_WS + 5 * TMOUT           # 1740
SM_TOT = SM_WT + TXF                # 2088


def _build_smalls(W, core, xtail_c):
    out = np.zeros((128, SM_TOT), dtype=np.float16)
    out[:, :SM_WS] = _build_wstack(W, core)
    out[:TKIN, SM_WS:SM_WT] = _build_wtail(W, core)
    out[:TKIN, SM_WT:] = xtail_c
    return out


def _build_wtail(W, core):
    """Tail stationary [120, 5*96] fp16: folded bands + SL + SR.

    S_w[g*TR + r, g*TM + m] = b4[w, r, m] for the 3 bands; SL only at
    g=0, SR only at g=TG-1 (their staging data is zero elsewhere, but
    zero coeffs keep it safe anyway)."""
    b4 = _build_block_mats(W, TM, False, core == NCORES - 1)  # [5, 10, 8]
    out = np.zeros((TKIN, 5, TMOUT), dtype=np.float16)
    for w in range(5):
        for g in range(TG):
            if w == 3 and g != 0:
                continue
            if w == 4 and g != TG - 1:
                continue
            out[g * TR : g * TR + TR, w, g * TM : g * TM + TM] = b4[w].astype(
                np.float16
            )
    return out.reshape(TKIN, 5 * TMOUT)


def _make_slabs(X16):
    """[8, SLAB, LEXT] fp16 slabs: staging cols 0..5 then X then 2 zero."""
    slabs = np.zeros((NCORES, SLAB, LEXT), dtype=np.float16)
    for c in range(NCORES):
        lo = c * RPC - 1
        hi = c * RPC + RPC + 1
        src_lo = max(0, lo)
        src_hi = min(H, hi)
        slabs[c, src_lo - lo : src_hi - lo, STG : STG + L] = X16[src_lo:src_hi, :]
    slabs[:, :, 0] = slabs[:, :, STG]          # X0
    slabs[:, :, 3] = slabs[:, :, STG + L - 1]  # XL
    return slabs


def _make_xtail(X16):
    """[8, TKIN, TXF] fp16 folded tail input, partition p = g*TR + r."""
    xt = np.zeros((NCORES, TKIN, TXF), dtype=np.float16)
    for c in range(NCORES):
        for r in range(TR):
            gr = c * RPC + MAIN - 1 + r
            if gr >= H:
                continue
            row = X16[gr]
            for g in range(TG):
                j0 = g * TGW - 1
                a = max(0, j0)
                b = min(L, j0 + TGW + 2)
                if a < b:
                    xt[c, g * TR + r, a - j0 : b - j0] = row[a:b]
            xt[c, 0 * TR + r, TGW + 2] = row[0]       # SL staging [X0, 0]
            xt[c, (TG - 1) * TR + r, TGW + 5] = row[L - 1]  # SR staging [0, XL]
    return xt


# ----------------------------------------------------------------------------
# Device program (SPMD; identical instruction stream on all 8 cores)
# ----------------------------------------------------------------------------

def build_nc(compile=True):
    nc = bacc.Bacc()
    x_d = nc.declare_dram_parameter("xslab", [SLAB, LEXT], F16, isOutput=False)
    sm_d = nc.declare_dram_parameter("smalls", [128, SM_TOT], F16, isOutput=False)
    y_d = nc.declare_dram_parameter("y", [MAIN, L], F16, isOutput=True)
    yt_d = nc.declare_dram_parameter("ytail", [TMOUT, TGW], F16, isOutput=True)

    with TileContext(nc) as tc:
        with (
            tc.tile_pool(name="wp", bufs=1) as wp,
            tc.tile_pool(name="xp", bufs=4) as xp,
            tc.tile_pool(name="yp", bufs=4) as yp,
            tc.tile_pool(name="pp", bufs=3, space="PSUM") as pp,
            tc.tile_pool(name="pt", bufs=1, space="PSUM") as ppt,
            tc.tile_pool(name="pw", bufs=1, space="PSUM") as ppw,
        ):
            smsb = wp.tile([128, SM_TOT], F16, name="smsb")
            wsb = smsb
            wz = wp.tile([128, 512], F16, name="wz")
            ytlsb = wp.tile([TMOUT, TGW], F16, name="ytlsb")

            # -- input DMA triggers, all up front (queues stream ahead) --
            xts = [
                xp.tile([128, LEXT], F16, name=f"xt{b}", tag="xt")
                for b in range(NBLK)
            ]
            # L half covers staging + X cols up to pair-1's v=+1 reach, so
            # pairs 0,1 of a block depend only on the L piece
            HALF = STG + 2050
            # Early per-queue rate is line-size sensitive (~100-180 GB/s
            # ramping; thin lines crawl).  Gating set (weights pack + block
            # 0) splits across both queues with fat lines only: smalls on
            # sync (spins up first), block 0 L+R on scalar.
            nc.sync.dma_start(out=smsb[:, :], in_=sm_d[:, :])
            nc.scalar.dma_start(out=xts[0][:, 0:HALF], in_=x_d[0:128, 0:HALF])
            nc.scalar.dma_start(
                out=xts[0][:, HALF:LEXT], in_=x_d[0:128, HALF:LEXT]
            )
            for b in range(1, NBLK):
                s = b * M
                nc.sync.dma_start(
                    out=xts[b][:, 0:HALF], in_=x_d[s : s + 128, 0:HALF]
                )
                nc.scalar.dma_start(
                    out=xts[b][:, HALF:LEXT], in_=x_d[s : s + 128, HALF:LEXT]
                )

            # -- PE warmup (p-state ramp); results are discarded --
            nc.vector.memset(wz[:, :], 0.0)
            pw = ppw.tile([128, 512], F32, name="pwarm")
            for i in range(NWARM):
                nc.tensor.matmul(
                    pw[0:128, 0:512], wz[0:128, 0:128], wz[0:128, 0:512],
                    start=True, stop=True,
                )

            def twm(w):
                c0 = SM_WS + w * TMOUT
                return smsb[0:TKIN, c0 : c0 + TMOUT]

            def txr(off, n):
                return smsb[0:TKIN, SM_WT + off : SM_WT + off + n]

            def do_tail():
                # column-folded tail; data rides in the packed smalls
                ptl = ppt.tile([128, TGW], F32, name="ptail")
                nc.tensor.matmul(
                    ptl[0:TMOUT, 0:TGW], twm(0), txr(0, TGW),
                    start=True, stop=False,
                )
                nc.tensor.matmul(
                    ptl[0:TMOUT, 0:TGW], twm(1), txr(1, TGW),
                    start=False, stop=False,
                )
                nc.tensor.matmul(
                    ptl[0:TMOUT, 0:TGW], twm(2), txr(2, TGW),
                    start=False, stop=False,
                )
                nc.tensor.matmul(
                    ptl[0:TMOUT, 0:2], twm(3), txr(TGW + 2, 2),
                    start=False, stop=False,
                )
                nc.tensor.matmul(
                    ptl[0:TMOUT, TGW - 10 : TGW - 8], twm(4), txr(TGW + 4, 2),
                    start=False, stop=True,
                )
                nc.vector.tensor_copy(ytlsb[0:TMOUT, 0:TGW], ptl[0:TMOUT, 0:TGW])
                nc.gpsimd.dma_start(out=yt_d[:, :], in_=ytlsb[0:TMOUT, 0:TGW])

            # -- main blocks (tail slots in after block 0) --
            for b in range(NBLK):
                s = b * M
                si = 0 if b == 0 else 1
                xt = xts[b]
                yt = yp.tile([128, L], F16, name=f"yt{b}", tag="yt")

                def wm(w):
                    base = (si * 5 + w) * WPAD_M
                    return wsb[0:128, base : base + M]

                def xr(c0, n):
                    # moving slice for X cols [c0, c0+n) (c0 may be -1 to
                    # touch the staged zero at tile col 5)
                    return xt[0:128, STG + c0 : STG + c0 + n]

                for k in range(NPAIR):
                    pt = pp.tile([128, PAIR], F32, name=f"pt{b}_{k}", tag="pt")
                    cA = k * PAIR
                    cB = cA + 512
                    # band v=0 (start clears each psum half)
                    nc.tensor.matmul(
                        pt[0:M, 0:512], wm(1), xr(cA, 512), start=True, stop=False
                    )
                    nc.tensor.matmul(
                        pt[0:M, 512:1024], wm(1), xr(cB, 512), start=True, stop=False
                    )
                    # band v=-1
                    nc.tensor.matmul(
                        pt[0:M, 0:512], wm(0), xr(cA - 1, 512),
                        start=False, stop=False,
                    )
                    nc.tensor.matmul(
                        pt[0:M, 512:1024], wm(0), xr(cB - 1, 512),
                        start=False, stop=False,
                    )
                    # band v=+1 (+ edge fixes on the outermost pairs)
                    nc.tensor.matmul(
                        pt[0:M, 0:512], wm(2), xr(cA + 1, 512),
                        start=False, stop=(k != 0),
                    )
                    if k == 0:
                        # col 0 fix: staging [X0, 0] at tile cols 0:2
                        nc.tensor.matmul(
                            pt[0:M, 0:2], wm(3), xt[0:128, 0:2],
                            start=False, stop=True,
                        )
                    nc.tensor.matmul(
                        pt[0:M, 512:1024], wm(2), xr(cB + 1, 512),
                        start=False, stop=(k != NPAIR - 1),
                    )
                    if k == NPAIR - 1:
                        # col L-1 fix: staging [0, XL] at tile cols 2:4
                        nc.tensor.matmul(
                            pt[0:M, 1022:1024], wm(4), xt[0:128, 2:4],
                            start=False, stop=True,
                        )
                    # evacuate pair to SBUF (fp32 -> fp16); the very last
                    # pair is split across both engines for tail latency
                    if b == NBLK - 1 and k == NPAIR - 1:
                        nc.scalar.copy(
                            yt[0:M, cA : cA + 512], pt[0:M, 0:512]
                        )
                        nc.vector.tensor_copy(
                            yt[0:M, cB : cB + 512], pt[0:M, 512:1024]
                        )
                        nc.scalar.dma_start(
                            out=y_d[s : s + M, cA : cA + 512],
                            in_=yt[0:M, cA : cA + 512],
                        )
                        nc.sync.dma_start(
                            out=y_d[s : s + M, cB : cB + 512],
                            in_=yt[0:M, cB : cB + 512],
                        )
                    else:
                        dst = yt[0:M, cA : cA + PAIR]
                        src = pt[0:M, 0:PAIR]
                        if k % 2 == 0:
                            nc.scalar.copy(dst, src)
                        else:
                            nc.vector.tensor_copy(dst, src)
                        if b == NBLK - 1:
                            # stream the last block's quarters out early,
                            # alternating queues
                            q_eng = nc.scalar if k % 2 == 0 else nc.sync
                            q_eng.dma_start(
                                out=y_d[s : s + M, cA : cA + PAIR],
                                in_=yt[0:M, cA : cA + PAIR],
                            )
                    # output DMA per block at k==3 below
                    if False:
                        pass
                    elif k == 3 and b < NBLK - 1:
                        # full-block output on the (warmed) hard queues;
                        # scalar carries more output since sync carries
                        # most of the input
                        if b == 0:
                            nc.sync.dma_start(
                                out=y_d[s : s + M, 0:L], in_=yt[0:M, 0:L]
                            )
                        elif b == 1:
                            nc.scalar.dma_start(
                                out=y_d[s : s + M, 0:L], in_=yt[0:M, 0:L]
                            )
                        else:
                            nc.sync.dma_start(
                                out=y_d[s : s + M, 0:L], in_=yt[0:M, 0:L]
                            )
                if b == 0:
                    do_tail()
    if compile:
        nc.compile()
    return nc


_NC_CACHE = None


def _get_nc():
    global _NC_CACHE
    if _NC_CACHE is None:
        _NC_CACHE = build_nc()
    return _NC_CACHE


def _run(X, W, trace=False, **spmd_kwargs):
    X16 = np.asarray(X, dtype=np.float16)
    slabs = _make_slabs(X16)
    xtails = _make_xtail(X16)
    in_maps = []
    for c in range(NCORES):
        in_maps.append(
            {
                "xslab": slabs[c],
                "smalls": _build_smalls(W, c, xtails[c]),
            }
        )
    res = run_bass_kernel_spmd(
        _get_nc(), in_maps, core_ids=list(range(NCORES)), trace=trace, **spmd_kwargs
    )
    Y = np.empty((H, L), dtype=np.float32)
    for c in range(NCORES):
        r0 = c * RPC
        Y[r0 : r0 + MAIN] = res.results[c]["y"].astype(np.float32)
        ytl = res.results[c]["ytail"].astype(np.float32)  # [96, 342]
        tail = ytl.reshape(TG, TM, TGW).transpose(1, 0, 2).reshape(TM, TG * TGW)
        Y[r0 + MAIN : r0 + RPC] = tail[:, :L]
    return Y, res


def kernel(X, W):
    Y, _ = _run(X, W)
    return Y


# revision 18
# speedup vs baseline: 1.1625x; 1.0405x over previous
"""Trainium2 Bass kernel for the ConvMod problem:

    Y1 = valid 2x2 cross-correlation(X, W)    # [4095, 4095]
    Y2 = transposed-conv(Y1, W)               # [4096, 4096]

The composite equals, in the interior, a 3x3 convolution of X with
K = corr(W, W), plus boundary corrections from the clipping of Y1's
domain (see _make_taps).

Distribution: data-parallel over rows across 8 cores; each core gets a
[514, 4104] fp16 row slab of X with a 1-row halo on each side, plus
per-core stationary band matrices, and produces its [512, 4096] slice
of Y2.  No collectives.  All HBM I/O is fp16 (the 2e-2 rel-err budget
has ~20x margin); PSUM accumulation is fp32.

Per core the 512 output rows split into 4 blocks of M=126 rows done as
tridiagonal band matmuls (3 column-offset passes over 4096 cols each,
PSUM-accumulated per 1024-col pair tile), plus an 8-row tail computed
in a column-folded layout [12 groups x 342 cols packed on partitions]
so its 3 band passes cost 342 moving columns instead of 4096.  Edge
corrections for output columns 0 / 4095 are N=2 matmuls on staging
columns; row-boundary corrections are baked into the per-core
stationary data (SPMD: same program, different data per core).

Engine roles: sync = input DMA (left halves + small tensors),
scalar = input DMA (right halves) + 2 PSUM evacuations per block,
vector = 2 evacuations per block + tail evac, gpsimd = output DMA
(SWDGE), tensor = matmuls only.  A few warmup matmuls on a zeroed
tile at t=0 climb the PE p-state ramp (0.65 -> 2.4 GHz) while the
first input DMA is in flight.
"""

import numpy as np

import concourse.bass as bass
from concourse import bacc
import concourse.mybir as mybir
from concourse.tile import TileContext
from concourse.bass_utils import run_bass_kernel_spmd

H = 4096
L = 4096
NCORES = 8
RPC = H // NCORES          # output rows per core: 512
SLAB = RPC + 2             # input slab rows per core (1-row halo each side)
STG = 6                    # staging cols at front: [X0, 0, 0, XL, 0, 0]
LEXT = STG + L + 2         # + 2 trailing zero cols (right pad for v=+1)
M = 126                    # output rows per main block
NBLK = 4                   # main blocks per core (4*126 = 504 rows)
MAIN = NBLK * M            # 504
PAIR = 1024                # psum pair-tile columns (2 banks)
NPAIR = L // PAIR          # 4
WPAD_M = 126
NSETS = 2                  # stationary sets: 0 = block 0, 1 = blocks 1..3
# tail: rows 504..511 in column-folded layout
TG = 12                    # groups
TGW = 342                  # cols per group (12*342 = 4104 >= 4096)
TR = 10                    # input rows for the tail (slab rows 504..513)
TM = 8                     # tail output rows
TKIN = TG * TR             # 120 moving partitions
TMOUT = TG * TM            # 96 output partitions
TXF = TGW + 2 + 4          # xtail free size: 344 window + 4 staging
NWARM = 14
F32 = mybir.dt.float32
F16 = mybir.dt.float16


# ----------------------------------------------------------------------------
# Host-side stationary-matrix construction
# ----------------------------------------------------------------------------

def _make_taps(W):
    W = np.asarray(W, dtype=np.float64)
    K = np.zeros((3, 3))
    for a in range(2):
        for b in range(2):
            for c in range(2):
                for d in range(2):
                    K[a - c + 1, b - d + 1] += W[a, b] * W[c, d]
    rowtop = np.zeros(3)
    rowbot = np.zeros(3)
    for b in range(2):
        for d in range(2):
            rowtop[b - d + 1] += W[1, b] * W[1, d]
            rowbot[b - d + 1] += W[0, b] * W[0, d]
    col0 = np.zeros(3)
    colL = np.zeros(3)
    for a in range(2):
        for c in range(2):
            col0[a - c + 1] += W[a, 1] * W[c, 1]
            colL[a - c + 1] += W[a, 0] * W[c, 0]
    corners = {
        (0, 0): W[1, 1] ** 2,
        (0, 1): W[1, 0] ** 2,
        (1, 0): W[0, 1] ** 2,
        (1, 1): W[0, 0] ** 2,
    }
    return K, rowtop, rowbot, col0, colL, corners


def _build_block_mats(W, Mb, first_row_global, last_row_global):
    """[5, Mb+2, Mb]: bands for v=-1,0,+1 then negated C0, C_L corrections."""
    K3, rowtop, rowbot, col0, colL, corners = _make_taps(W)
    Kin = Mb + 2
    mats = np.zeros((5, Kin, Mb))
    for m in range(Mb):
        for u in (-1, 0, 1):
            k = m + 1 + u
            for vi, v in enumerate((-1, 0, 1)):
                mats[vi, k, m] = K3[u + 1, v + 1]
            mats[3, k, m] = -col0[u + 1]
            mats[4, k, m] = -colL[u + 1]
    if first_row_global:
        for vi, v in enumerate((-1, 0, 1)):
            mats[vi, 1, 0] -= rowtop[v + 1]
        mats[3, 1, 0] += corners[(0, 0)]
        mats[4, 1, 0] += corners[(0, 1)]
    if last_row_global:
        m = Mb - 1
        for vi, v in enumerate((-1, 0, 1)):
            mats[vi, m + 1, m] -= rowbot[v + 1]
        mats[3, m + 1, m] += corners[(1, 0)]
        mats[4, m + 1, m] += corners[(1, 1)]
    return mats


def _build_wstack(W, core):
    """Per-core stationary stack [128, 10*126] fp16 (set-major, k-major)."""
    out = np.zeros((128, NSETS, 5, WPAD_M), dtype=np.float16)
    b0 = _build_block_mats(W, M, core == 0, False)
    mid = _build_block_mats(W, M, False, False)
    for w in range(5):
        out[:128, 0, w, :M] = b0[w].astype(np.float16)
        out[:128, 1, w, :M] = mid[w].astype(np.float16)
    return out.reshape(128, NSETS * 5 * WPAD_M)


# packed [128, SM_TOT] layout: wstack | wtail | xtail (fat DMA lines)
SM_WS = NSETS * 5 * WPAD_M          # 1260
SM_WT = SM_WS + 5 * TMOUT           # 1740
SM_TOT = SM_WT + TXF                # 2088


def _build_smalls(W, core, xtail_c):
    out = np.zeros((128, SM_TOT), dtype=np.float16)
    out[:, :SM_WS] = _build_wstack(W, core)
    out[:TKIN, SM_WS:SM_WT] = _build_wtail(W, core)
    out[:TKIN, SM_WT:] = xtail_c
    return out


def _build_wtail(W, core):
    """Tail stationary [120, 5*96] fp16: folded bands + SL + SR.

    S_w[g*TR + r, g*TM + m] = b4[w, r, m] for the 3 bands; SL only at
    g=0, SR only at g=TG-1 (their staging data is zero elsewhere, but
    zero coeffs keep it safe anyway)."""
    b4 = _build_block_mats(W, TM, False, core == NCORES - 1)  # [5, 10, 8]
    out = np.zeros((TKIN, 5, TMOUT), dtype=np.float16)
    for w in range(5):
        for g in range(TG):
            if w == 3 and g != 0:
                continue
            if w == 4 and g != TG - 1:
                continue
            out[g * TR : g * TR + TR, w, g * TM : g * TM + TM] = b4[w].astype(
                np.float16
            )
    return out.reshape(TKIN, 5 * TMOUT)


def _make_slabs(X16):
    """[8, SLAB, LEXT] fp16 slabs: staging cols 0..5 then X then 2 zero."""
    slabs = np.zeros((NCORES, SLAB, LEXT), dtype=np.float16)
    for c in range(NCORES):
        lo = c * RPC - 1
        hi = c * RPC + RPC + 1
        src_lo = max(0, lo)
        src_hi = min(H, hi)
        slabs[c, src_lo - lo : src_hi - lo, STG : STG + L] = X16[src_lo:src_hi, :]
    slabs[:, :, 0] = slabs[:, :, STG]          # X0
    slabs[:, :, 3] = slabs[:, :, STG + L - 1]  # XL
    return slabs


def _make_xtail(X16):
    """[8, TKIN, TXF] fp16 folded tail input, partition p = g*TR + r."""
    xt = np.zeros((NCORES, TKIN, TXF), dtype=np.float16)
    for c in range(NCORES):
        for r in range(TR):
            gr = c * RPC + MAIN - 1 + r
            if gr >= H:
                continue
            row = X16[gr]
            for g in range(TG):
                j0 = g * TGW - 1
                a = max(0, j0)
                b = min(L, j0 + TGW + 2)
                if a < b:
                    xt[c, g * TR + r, a - j0 : b - j0] = row[a:b]
            xt[c, 0 * TR + r, TGW + 2] = row[0]       # SL staging [X0, 0]
            xt[c, (TG - 1) * TR + r, TGW + 5] = row[L - 1]  # SR staging [0, XL]
    return xt


# ----------------------------------------------------------------------------
# Device program (SPMD; identical instruction stream on all 8 cores)
# ----------------------------------------------------------------------------

def build_nc(compile=True):
    nc = bacc.Bacc()
    x_d = nc.declare_dram_parameter("xslab", [SLAB, LEXT], F16, isOutput=False)
    sm_d = nc.declare_dram_parameter("smalls", [128, SM_TOT], F16, isOutput=False)
    y_d = nc.declare_dram_parameter("y", [MAIN, L], F16, isOutput=True)
    yt_d = nc.declare_dram_parameter("ytail", [TMOUT, TGW], F16, isOutput=True)

    with TileContext(nc) as tc:
        with (
            tc.tile_pool(name="wp", bufs=1) as wp,
            tc.tile_pool(name="xp", bufs=4) as xp,
            tc.tile_pool(name="yp", bufs=4) as yp,
            tc.tile_pool(name="pp", bufs=3, space="PSUM") as pp,
            tc.tile_pool(name="pt", bufs=1, space="PSUM") as ppt,
            tc.tile_pool(name="pw", bufs=1, space="PSUM") as ppw,
        ):
            smsb = wp.tile([128, SM_TOT], F16, name="smsb")
            wsb = smsb
            wz = wp.tile([128, 512], F16, name="wz")
            ytlsb = wp.tile([TMOUT, TGW], F16, name="ytlsb")

            # -- input DMA triggers, all up front (queues stream ahead) --
            xts = [
                xp.tile([128, LEXT], F16, name=f"xt{b}", tag="xt")
                for b in range(NBLK)
            ]
            # L half covers staging + X cols up to pair-1's v=+1 reach, so
            # pairs 0,1 of a block depend only on the L piece
            HALF = STG + 2050
            # Early per-queue rate is line-size sensitive (~100-180 GB/s
            # ramping; thin lines crawl).  Gating set (weights pack + block
            # 0) splits across both queues with fat lines only: smalls on
            # sync (spins up first), block 0 L+R on scalar.
            nc.sync.dma_start(out=smsb[:, :], in_=sm_d[:, :])
            nc.scalar.dma_start(out=xts[0][:, 0:HALF], in_=x_d[0:128, 0:HALF])
            nc.sync.dma_start(
                out=xts[0][:, HALF:LEXT], in_=x_d[0:128, HALF:LEXT]
            )
            for b in range(1, NBLK):
                s = b * M
                nc.sync.dma_start(
                    out=xts[b][:, 0:HALF], in_=x_d[s : s + 128, 0:HALF]
                )
                nc.scalar.dma_start(
                    out=xts[b][:, HALF:LEXT], in_=x_d[s : s + 128, HALF:LEXT]
                )

            # -- PE warmup (p-state ramp); results are discarded --
            nc.vector.memset(wz[:, :], 0.0)
            pw = ppw.tile([128, 512], F32, name="pwarm")
            for i in range(NWARM):
                nc.tensor.matmul(
                    pw[0:128, 0:512], wz[0:128, 0:128], wz[0:128, 0:512],
                    start=True, stop=True,
                )

            def twm(w):
                c0 = SM_WS + w * TMOUT
                return smsb[0:TKIN, c0 : c0 + TMOUT]

            def txr(off, n):
                return smsb[0:TKIN, SM_WT + off : SM_WT + off + n]

            def do_tail():
                # column-folded tail; data rides in the packed smalls
                ptl = ppt.tile([128, TGW], F32, name="ptail")
                nc.tensor.matmul(
                    ptl[0:TMOUT, 0:TGW], twm(0), txr(0, TGW),
                    start=True, stop=False,
                )
                nc.tensor.matmul(
                    ptl[0:TMOUT, 0:TGW], twm(1), txr(1, TGW),
                    start=False, stop=False,
                )
                nc.tensor.matmul(
                    ptl[0:TMOUT, 0:TGW], twm(2), txr(2, TGW),
                    start=False, stop=False,
                )
                nc.tensor.matmul(
                    ptl[0:TMOUT, 0:2], twm(3), txr(TGW + 2, 2),
                    start=False, stop=False,
                )
                nc.tensor.matmul(
                    ptl[0:TMOUT, TGW - 10 : TGW - 8], twm(4), txr(TGW + 4, 2),
                    start=False, stop=True,
                )
                nc.vector.tensor_copy(ytlsb[0:TMOUT, 0:TGW], ptl[0:TMOUT, 0:TGW])
                nc.gpsimd.dma_start(out=yt_d[:, :], in_=ytlsb[0:TMOUT, 0:TGW])

            # -- main blocks (tail slots in after block 0) --
            for b in range(NBLK):
                s = b * M
                si = 0 if b == 0 else 1
                xt = xts[b]
                yt = yp.tile([128, L], F16, name=f"yt{b}", tag="yt")

                def wm(w):
                    base = (si * 5 + w) * WPAD_M
                    return wsb[0:128, base : base + M]

                def xr(c0, n):
                    # moving slice for X cols [c0, c0+n) (c0 may be -1 to
                    # touch the staged zero at tile col 5)
                    return xt[0:128, STG + c0 : STG + c0 + n]

                for k in range(NPAIR):
                    if b == 0 and k == 2:
                        # slot the tiny folded tail here: it absorbs any
                        # wait for block 0's R half
                        do_tail()
                    pt = pp.tile([128, PAIR], F32, name=f"pt{b}_{k}", tag="pt")
                    cA = k * PAIR
                    cB = cA + 512
                    # band v=0 (start clears each psum half)
                    nc.tensor.matmul(
                        pt[0:M, 0:512], wm(1), xr(cA, 512), start=True, stop=False
                    )
                    nc.tensor.matmul(
                        pt[0:M, 512:1024], wm(1), xr(cB, 512), start=True, stop=False
                    )
                    # band v=-1
                    nc.tensor.matmul(
                        pt[0:M, 0:512], wm(0), xr(cA - 1, 512),
                        start=False, stop=False,
                    )
                    nc.tensor.matmul(
                        pt[0:M, 512:1024], wm(0), xr(cB - 1, 512),
                        start=False, stop=False,
                    )
                    # band v=+1 (+ edge fixes on the outermost pairs)
                    nc.tensor.matmul(
                        pt[0:M, 0:512], wm(2), xr(cA + 1, 512),
                        start=False, stop=(k != 0),
                    )
                    if k == 0:
                        # col 0 fix: staging [X0, 0] at tile cols 0:2
                        nc.tensor.matmul(
                            pt[0:M, 0:2], wm(3), xt[0:128, 0:2],
                            start=False, stop=True,
                        )
                    nc.tensor.matmul(
                        pt[0:M, 512:1024], wm(2), xr(cB + 1, 512),
                        start=False, stop=(k != NPAIR - 1),
                    )
                    if k == NPAIR - 1:
                        # col L-1 fix: staging [0, XL] at tile cols 2:4
                        nc.tensor.matmul(
                            pt[0:M, 1022:1024], wm(4), xt[0:128, 2:4],
                            start=False, stop=True,
                        )
                    # evacuate pair to SBUF (fp32 -> fp16); the very last
                    # pair is split across both engines for tail latency
                    if b == NBLK - 1 and k == NPAIR - 1:
                        nc.scalar.copy(
                            yt[0:M, cA : cA + 512], pt[0:M, 0:512]
                        )
                        nc.vector.tensor_copy(
                            yt[0:M, cB : cB + 512], pt[0:M, 512:1024]
                        )
                        nc.scalar.dma_start(
                            out=y_d[s : s + M, cA : cA + 512],
                            in_=yt[0:M, cA : cA + 512],
                        )
                        nc.sync.dma_start(
                            out=y_d[s : s + M, cB : cB + 512],
                            in_=yt[0:M, cB : cB + 512],
                        )
                    else:
                        dst = yt[0:M, cA : cA + PAIR]
                        src = pt[0:M, 0:PAIR]
                        if k % 2 == 0:
                            nc.scalar.copy(dst, src)
                        else:
                            nc.vector.tensor_copy(dst, src)
                        if b == NBLK - 1:
                            # stream the last block's quarters out early,
                            # alternating queues
                            q_eng = nc.scalar if k % 2 == 0 else nc.sync
                            q_eng.dma_start(
                                out=y_d[s : s + M, cA : cA + PAIR],
                                in_=yt[0:M, cA : cA + PAIR],
                            )
                    # output DMA per block at k==3 below
                    if False:
                        pass
                    elif k == 3 and b < NBLK - 1:
                        # full-block output on the (warmed) hard queues;
                        # scalar carries more output since sync carries
                        # most of the input
                        if b == 0:
                            nc.sync.dma_start(
                                out=y_d[s : s + M, 0:L], in_=yt[0:M, 0:L]
                            )
                        elif b == 1:
                            nc.scalar.dma_start(
                                out=y_d[s : s + M, 0:L], in_=yt[0:M, 0:L]
                            )
                        else:
                            nc.sync.dma_start(
                                out=y_d[s : s + M, 0:L], in_=yt[0:M, 0:L]
                            )

    if compile:
        nc.compile()
    return nc


_NC_CACHE = None


def _get_nc():
    global _NC_CACHE
    if _NC_CACHE is None:
        _NC_CACHE = build_nc()
    return _NC_CACHE


def _run(X, W, trace=False, **spmd_kwargs):
    X16 = np.asarray(X, dtype=np.float16)
    slabs = _make_slabs(X16)
    xtails = _make_xtail(X16)
    in_maps = []
    for c in range(NCORES):
        in_maps.append(
            {
                "xslab": slabs[c],
                "smalls": _build_smalls(W, c, xtails[c]),
            }
        )
    res = run_bass_kernel_spmd(
        _get_nc(), in_maps, core_ids=list(range(NCORES)), trace=trace, **spmd_kwargs
    )
    Y = np.empty((H, L), dtype=np.float32)
    for c in range(NCORES):
        r0 = c * RPC
        Y[r0 : r0 + MAIN] = res.results[c]["y"].astype(np.float32)
        ytl = res.results[c]["ytail"].astype(np.float32)  # [96, 342]
        tail = ytl.reshape(TG, TM, TGW).transpose(1, 0, 2).reshape(TM, TG * TGW)
        Y[r0 + MAIN : r0 + RPC] = tail[:, :L]
    return Y, res


def kernel(X, W):
    Y, _ = _run(X, W)
    return Y
